# Initial kernel scaffold
#
"""Distributed Trainium2 kernel for nn_Attention (dense transformer block:
fused QKV projection + per-head RMSNorm + rotary + causal GQA attention + output
projection), running SPMD on 8 NeuronCores.

Sharding (rank-uniform, no divergent control flow):
  - 8 cores = 2 batch groups x 4 tensor-parallel ranks.
  - Core c: batch b = c // 4, rank r = c % 4.
  - QKV projection + attention are head-sharded: core r computes q heads
    4r..4r+3 and kv head r for ALL tokens of its batch (wqkv column slice is
    per-core input data, so the compiled graph is identical on every core).
  - Every core runs the same causal tile sweep -> perfect load balance.
  - One AllToAll (2MB bf16) re-shards y from head-split to token-split, then
    the output projection runs locally with the full contraction dim
    (no all-reduce).

Layout tricks:
  - Host pre-transposes x, wqkv, wo so the kernel's matmuls need no on-device
    transposes (except tiny 128x128 PE transposes for V).
  - Scores are computed transposed [kv, q] so the softmax denominator comes
    from a ones-vector matmul on the TensorEngine and exp is fused into the
    PSUM->SBUF eviction on the ScalarEngine.
  - RMSNorm reduces to a per-token scalar (applied to q pre-matmul, and to k
    pre-matmul), computed with a ones-matmul over the squared tile; the
    1/sqrt(head_dim) score scale folds into the q-side scalar.
  - Rope's even/odd pair swap is a 128x128 permutation matmul in f32r.
  - All big matmuls run in bf16 with f32 PSUM accumulation.
"""

import numpy as np
import ml_dtypes

import concourse.bass as bass
import concourse.bass_isa as bass_isa
import concourse.mybir as mybir
import concourse.tile as tile
from concourse import bacc
from concourse.bass_utils import run_bass_kernel_spmd

BF16 = mybir.dt.bfloat16
F32 = mybir.dt.float32
F32R = mybir.dt.float32r

DIM = 2048
NH = 16
NKV = 4
HD = 128
EPS = 1e-5
N_CORES = 8
RG = [[0, 1, 2, 3], [4, 5, 6, 7]]  # per-batch tensor-parallel groups

HL = NH // NKV  # q heads per core (= GQA group size) = 4
EW = HL * HD + 2 * HD  # wqkv column-slice width per core = 768
NDT = DIM // 128  # contraction tiles = 16


def build_graph(S):
    """Build + compile the SPMD graph for sequence length S. Returns nc."""
    TPT = S // 4       # tokens per core after the A2A (output rows per core)
    TCW = S // 4       # token chunk width for phase 1 (moving dim <= 512)
    NTT = S // TCW     # number of token chunks = 4
    QC = 512           # attention q-chunk width
    KB = 128           # kv block size
    NQC = S // QC      # q chunks per head
    NB = S // 128      # 128-token blocks (for V layout)

    nc = bacc.Bacc("TRN2", target_bir_lowering=False, debug=False,
                   num_devices=N_CORES)

    # ---- DRAM I/O ----
    xT_d = nc.dram_tensor("xT", [DIM, S], BF16, kind="ExternalInput")
    w_d = nc.dram_tensor("wslice", [DIM, EW], BF16, kind="ExternalInput")
    wo_d = nc.dram_tensor("woT", [DIM, DIM], BF16, kind="ExternalInput")
    cos_d = nc.dram_tensor("cosF", [128, S], F32, kind="ExternalInput")
    sin_d = nc.dram_tensor("sinF", [128, S], F32, kind="ExternalInput")
    swp_d = nc.dram_tensor("swapP", [128, 128], F32, kind="ExternalInput")
    idn_d = nc.dram_tensor("ident", [128, 128], BF16, kind="ExternalInput")
    msk_d = nc.dram_tensor("masks", [KB, (QC // KB) * QC], BF16, kind="ExternalInput")
    oh_d = nc.dram_tensor("onehots", [128, (S // 512) * (S // 512)], BF16,
                          kind="ExternalInput")
    sel_d = nc.dram_tensor("sels", [S // 512, (S // 512) * 128], BF16,
                           kind="ExternalInput")
    qw_d = nc.dram_tensor("qw", [128, 1], F32, kind="ExternalInput")
    kw_d = nc.dram_tensor("kw", [128, 1], F32, kind="ExternalInput")
    out_d = nc.dram_tensor("out", [DIM, TPT], BF16, kind="ExternalOutput")

    with tile.TileContext(nc) as tc:
        with tc.tile_pool(name="const", bufs=1) as cpool, \
             tc.tile_pool(name="wq", bufs=1) as wpool, \
             tc.tile_pool(name="big", bufs=1) as bigpool, \
             tc.tile_pool(name="dram", bufs=1, space="DRAM") as dpool:

            # constants
            swp = cpool.tile([128, 128], F32R, tag="swp")
            nc.sync.dma_start(swp[:], swp_d[:].bitcast(F32R))
            idn = cpool.tile([128, 128], BF16, tag="idn")
            nc.sync.dma_start(idn[:], idn_d[:])
            msk = cpool.tile([KB, (QC // KB) * QC], BF16, tag="msk")
            nc.sync.dma_start(msk[:], msk_d[:])
            qw = cpool.tile([128, 1], F32, tag="qw")
            nc.sync.dma_start(qw[:], qw_d[:])
            kw = cpool.tile([128, 1], F32, tag="kw")
            nc.sync.dma_start(kw[:], kw_d[:])
            ones = cpool.tile([128, 1], BF16, tag="ones")
            nc.vector.memset(ones[:], 1.0)
            oneh = cpool.tile([128, NQC * NQC], BF16, tag="oneh")
            nc.sync.dma_start(oneh[:], oh_d[:])
            sel = cpool.tile([NQC, NQC * 128], BF16, tag="sel")
            nc.sync.dma_start(sel[:], sel_d[:])
            onec = cpool.tile([1, 128], BF16, tag="onec")
            nc.vector.memset(onec[:], 1.0)
            b0 = cpool.tile([128, 1], F32, tag="b0")
            nc.vector.memset(b0[:], 0.0)
            bq = cpool.tile([1, 1], F32, tag="bq")
            nc.vector.memset(bq[:], float(HD * EPS))
            bk = cpool.tile([1, 1], F32, tag="bk")
            nc.vector.memset(bk[:], float(EPS))

            # full wqkv slice, staged once: [128, dt*EW + e]
            # (one DMA per contraction tile so the first matmuls start early)
            w_sb = wpool.tile([128, NDT * EW], BF16, tag="w")

            # long-lived activations
            qT = bigpool.tile([128, HL * S], BF16, tag="qT")
            kT = bigpool.tile([128, S], BF16, tag="kT")
            V = bigpool.tile([128, S], BF16, tag="V")   # [tok%128, blk*128+d]
            yT = bigpool.tile([128, HL * S], BF16, tag="yT")

            # ---------------- Phase 1: QKV + norm + rope ----------------
            with tc.tile_pool(name="x", bufs=2) as xpool, \
                 tc.tile_pool(name="cs", bufs=2) as cspool, \
                 tc.tile_pool(name="scr", bufs=2) as scr, \
                 tc.tile_pool(name="smol", bufs=2) as smol, \
                 tc.tile_pool(name="p1", bufs=2, space="PSUM") as p1, \
                 tc.tile_pool(name="psw", bufs=2, space="PSUM") as psw, \
                 tc.tile_pool(name="pss", bufs=2, space="PSUM") as pss, \
                 tc.tile_pool(name="pvt", bufs=2, space="PSUM") as pvt:

                def process_qk(ps, et, tt, cos_t, sin_t):
                    is_q = et < HL
                    # squared tile (raw, pre-normweight) -> bf16
                    sqv = smol.tile([128, TCW], BF16, tag="sq2", name="sqv")
                    nc.scalar.activation(
                        sqv[:], ps[:],
                        mybir.ActivationFunctionType.Square, bias=b0[:])
                    ss = pss.tile([1, TCW], F32, tag="ss", name="ss")
                    nc.tensor.matmul(ss[:], ones[:], sqv[:],
                                     start=True, stop=True)
                    qf = scr.tile([128, TCW], F32R, tag="qf", name="qf")
                    nc.scalar.mul(qf[:], ps[:], (qw if is_q else kw)[:])
                    sq = smol.tile([1, TCW], F32, tag="sqs", name="sq")
                    if is_q:
                        nc.scalar.activation(
                            sq[:], ss[:],
                            mybir.ActivationFunctionType.Sqrt,
                            bias=bq[:], scale=1.0)
                    else:
                        nc.scalar.activation(
                            sq[:], ss[:],
                            mybir.ActivationFunctionType.Sqrt,
                            bias=bk[:], scale=1.0 / HD)
                    inv = smol.tile([1, TCW], F32, tag="inv", name="inv")
                    nc.vector.reciprocal_approx_fast(inv[:], sq[:])
                    invb = scr.tile([128, TCW], F32, tag="invb", name="invb")
                    nc.gpsimd.partition_broadcast(invb[:], inv[:])
                    sw = psw.tile([128, TCW], F32, tag="sw", name="sw")
                    nc.tensor.matmul(sw[:], swp[:], qf[:],
                                     start=True, stop=True)
                    t1 = scr.tile([128, TCW], F32, tag="t1", name="t1")
                    nc.vector.tensor_mul(t1[:], qf[:], cos_t[:])
                    t2 = scr.tile([128, TCW], F32, tag="t2", name="t2")
                    nc.vector.tensor_mul(t2[:], sw[:], sin_t[:])
                    nc.vector.tensor_add(t1[:], t1[:], t2[:])
                    dst = (qT[:, et * S + tt * TCW: et * S + tt * TCW + TCW]
                           if is_q else
                           kT[:, tt * TCW: tt * TCW + TCW])
                    nc.vector.tensor_mul(dst, t1[:], invb[:])

                def process_v(ps, tt):
                    vb = smol.tile([128, TCW], BF16, tag="vb", name="vb")
                    nc.scalar.copy(vb[:], ps[:])
                    for bb in range(TCW // 128):
                        tp = pvt.tile([128, 128], BF16, tag="tp", name="tp")
                        nc.tensor.transpose(
                            tp[:], vb[:, bb * 128:(bb + 1) * 128], idn[:])
                        blk = tt * (TCW // 128) + bb
                        nc.scalar.copy(V[:, blk * 128:(blk + 1) * 128], tp[:])

                pending = None  # (psum, et, tt, cos_t, sin_t)
                for tt in range(NTT):
                    xt = xpool.tile([128, NDT * TCW], BF16, tag="x")
                    for dt in range(NDT):
                        if tt == 0:  # interleave weight panels in need-order
                            nc.sync.dma_start(
                                w_sb[:, dt * EW:(dt + 1) * EW],
                                w_d[dt * 128:(dt + 1) * 128, :])
                        nc.scalar.dma_start(
                            xt[:, dt * TCW:(dt + 1) * TCW],
                            xT_d[dt * 128:(dt + 1) * 128,
                                 tt * TCW:(tt + 1) * TCW])
                    cos_t = cspool.tile([128, TCW], F32, tag="cos")
                    nc.sync.dma_start(cos_t[:], cos_d[:, tt * TCW:(tt + 1) * TCW])
                    sin_t = cspool.tile([128, TCW], F32, tag="sin")
                    nc.sync.dma_start(sin_t[:], sin_d[:, tt * TCW:(tt + 1) * TCW])

                    for et in range(HL + 2):
                        ps = p1.tile([128, TCW], F32, tag="ps")
                        for dt in range(NDT):
                            nc.tensor.matmul(
                                ps[:],
                                w_sb[:, dt * EW + et * 128:dt * EW + (et + 1) * 128],
                                xt[:, dt * TCW:(dt + 1) * TCW],
                                start=(dt == 0), stop=(dt == NDT - 1),
                            )
                        # process the PREVIOUS tile now: its cross-engine
                        # waits overlap this tile's matmul group
                        if pending is not None:
                            pps, pet, ptt, pc, psn_ = pending
                            if pet < HL + 1:
                                process_qk(pps, pet, ptt, pc, psn_)
                            else:
                                process_v(pps, ptt)
                        pending = (ps, et, tt, cos_t, sin_t)
                pps, pet, ptt, pc, psn_ = pending
                if pet < HL + 1:
                    process_qk(pps, pet, ptt, pc, psn_)
                else:
                    process_v(pps, ptt)

            # ---------------- Phase 2: causal attention ----------------
            with tc.tile_pool(name="exp", bufs=6) as epool, \
                 tc.tile_pool(name="rs", bufs=2) as rspool, \
                 tc.tile_pool(name="pa", bufs=2, space="PSUM") as pa, \
                 tc.tile_pool(name="py", bufs=1, space="PSUM") as py, \
                 tc.tile_pool(name="pn", bufs=1, space="PSUM") as pn, \
                 tc.tile_pool(name="pb", bufs=1, space="PSUM") as pb:

                # yf gathers the A2A'd y slices as they arrive, per head
                yf = bigpool.tile([128, NDT * TPT], BF16, tag="yf")
                pid = nc.gpsimd.partition_id()
                roff = (pid % 4) * TPT

                for h in range(HL):
                    # g-outer sweep: each kv block's kT/V stationary serves all
                    # active q-chunks before the PE loads the next weights.
                    ps_ys = [py.tile([128, QC], F32, tag=f"y{qc}",
                                     name=f"psy{qc}")
                             for qc in range(NQC)]
                    psn = pn.tile([NQC, QC], F32, tag="n")
                    for g in range(NB):
                        qcs = [qc for qc in range(NQC) if g < 4 * (qc + 1)]
                        exs = {}
                        for qc in qcs:
                            ps_s = pa.tile([KB, QC], F32, tag="s")
                            nc.tensor.matmul(
                                ps_s[:],
                                kT[:, g * KB:(g + 1) * KB],
                                qT[:, h * S + qc * QC: h * S + (qc + 1) * QC],
                                start=True, stop=True)
                            ex = epool.tile([KB, QC], BF16, tag="e")
                            nc.scalar.activation(
                                ex[:], ps_s[:],
                                mybir.ActivationFunctionType.Exp,
                                bias=b0[0:KB, :])
                            if qc == g // 4:  # diagonal region: causal mask
                                t = g % 4
                                nc.vector.tensor_mul(
                                    ex[:], ex[:], msk[:, t * QC:(t + 1) * QC])
                            exs[qc] = ex
                        for qc in qcs:
                            nc.tensor.matmul(
                                ps_ys[qc][:],
                                V[:, g * 128:(g + 1) * 128],
                                exs[qc][:],
                                start=(g == 0), stop=(g == 4 * qc + 3))
                        for qc in qcs:
                            nc.tensor.matmul(
                                psn[:], oneh[:, qc * NQC:(qc + 1) * NQC],
                                exs[qc][:],
                                start=(g == 0 and qc == 0),
                                stop=(g == NB - 1 and qc == NQC - 1))

                    rs4 = rspool.tile([NQC, QC], F32, tag="r")
                    nc.vector.reciprocal_approx_fast(rs4[:], psn[:])
                    rsb16 = rspool.tile([NQC, QC], BF16, tag="r16")
                    nc.vector.tensor_copy(rsb16[:], rs4[:])
                    for qd in range(NQC):
                        ps_b = pb.tile([128, QC], F32, tag="b")
                        nc.tensor.matmul(
                            ps_b[:], sel[:, qd * 128:(qd + 1) * 128],
                            rsb16[:], start=True, stop=True)
                        rsb = rspool.tile([128, QC], F32, tag="rb")
                        nc.scalar.copy(rsb[:], ps_b[:])
                        nc.vector.tensor_mul(
                            yT[:, h * S + qd * QC: h * S + (qd + 1) * QC],
                            ps_ys[qd][:], rsb[:])

                    # per-head AllGather of y, overlapped with the next head's
                    # attention; each core reads back its token quarter via a
                    # partition_id-derived dynamic column offset.
                    in_b = dpool.tile([128, S], BF16, tag=f"agin{h}")
                    out_b = dpool.tile([4 * 128, S], BF16, tag=f"agout{h}")
                    nc.scalar.dma_start(in_b[:], yT[:, h * S:(h + 1) * S])
                    nc.gpsimd.collective_compute(
                        "AllGather", mybir.AluOpType.bypass,
                        replica_groups=RG,
                        ins=[in_b.opt()], outs=[out_b.opt()])
                    for r in range(4):
                        et = 4 * r + h
                        nc.gpsimd.dma_start(
                            yf[:, et * TPT:(et + 1) * TPT],
                            out_b[r * 128:(r + 1) * 128, bass.ds(roff, TPT)])

            # ---------------- output projection (two passes) ----------------
            # Pass A accumulates heads 0..2 (available before the last
            # AllGather) into bf16 partials; pass B adds head 3's contribution
            # as soon as its gather lands. woT streamed per output tile.
            with tc.tile_pool(name="wos", bufs=4) as wopool, \
                 tc.tile_pool(name="part", bufs=1) as partpool, \
                 tc.tile_pool(name="ot", bufs=2) as otpool, \
                 tc.tile_pool(name="po", bufs=2, space="PSUM") as po:
                part = partpool.tile([128, NDT * TPT], BF16, tag="part")
                etsA = [4 * r + hl for hl in range(HL - 1) for r in range(4)]
                etsB = [4 * r + (HL - 1) for r in range(4)]
                for ot in range(NDT):
                    wos = wopool.tile([128, NDT * 128], BF16, tag="wos")
                    nc.sync.dma_start(
                        wos[:].rearrange("p (a o) -> p a o", a=NDT),
                        wo_d[:, ot * 128:(ot + 1) * 128]
                            .rearrange("(a p) o -> p a o", p=128))
                    ps_o = po.tile([128, TPT], F32, tag="o")
                    for i, et in enumerate(etsA):
                        nc.tensor.matmul(
                            ps_o[:],
                            wos[:, et * 128:(et + 1) * 128],
                            yf[:, et * TPT:(et + 1) * TPT],
                            start=(i == 0), stop=(i == len(etsA) - 1))
                    nc.scalar.copy(part[:, ot * TPT:(ot + 1) * TPT], ps_o[:])
                for ot in range(NDT):
                    wosb = wopool.tile([128, 4 * 128], BF16, tag="wosb")
                    for i, et in enumerate(etsB):
                        nc.sync.dma_start(
                            wosb[:, i * 128:(i + 1) * 128],
                            wo_d[et * 128:(et + 1) * 128,
                                 ot * 128:(ot + 1) * 128])
                    ps_o = po.tile([128, TPT], F32, tag="o")
                    for i, et in enumerate(etsB):
                        nc.tensor.matmul(
                            ps_o[:],
                            wosb[:, i * 128:(i + 1) * 128],
                            yf[:, et * TPT:(et + 1) * TPT],
                            start=(i == 0), stop=(i == len(etsB) - 1))
                    ott = otpool.tile([128, TPT], BF16, tag="ot")
                    nc.vector.tensor_add(ott[:], ps_o[:],
                                         part[:, ot * TPT:(ot + 1) * TPT])
                    nc.sync.dma_start(out_d[ot * 128:(ot + 1) * 128, :], ott[:])

    nc.compile()
    return nc


def make_in_maps(x, freqs_cis, wqkv, wo, q_norm_w, k_norm_w, S):
    """Host-side sharding / layout prep. Returns list of 8 input dicts."""
    bf = ml_dtypes.bfloat16
    QC = 512
    KB = 128

    # rope tables: [128, S]; row 2i & 2i+1 carry cos[t, i]; sin signed
    cos = np.asarray(freqs_cis[:S, :, 0], np.float32)   # [S, 64]
    sin = np.asarray(freqs_cis[:S, :, 1], np.float32)
    cosF = np.repeat(cos.T, 2, axis=0).astype(np.float32)      # [128, S]
    sinF = np.repeat(sin.T, 2, axis=0).astype(np.float32)
    sinF[0::2] *= -1.0
    cosF = np.ascontiguousarray(cosF)
    sinF = np.ascontiguousarray(sinF)

    swapP = np.zeros((128, 128), np.float32)
    for i in range(64):
        swapP[2 * i, 2 * i + 1] = 1.0
        swapP[2 * i + 1, 2 * i] = 1.0
    ident = np.eye(128, dtype=bf)

    # masks [KB, (QC//KB)*QC]: pattern t for the t-th kv block inside the
    # diagonal QC-region: allowed iff (t*KB + r) <= c
    r = np.arange(KB)[:, None]
    c = np.arange(QC)[None, :]
    pats = [((t * KB + r) <= c).astype(np.float32) for t in range(QC // KB)]
    masks = np.concatenate(pats, axis=1).astype(bf)

    NQC = S // 512
    oneh = np.zeros((128, NQC * NQC), np.float32)
    for qc in range(NQC):
        oneh[:, qc * NQC + qc] = 1.0
    oneh = oneh.astype(bf)
    sels = np.zeros((NQC, NQC * 128), np.float32)
    for qd in range(NQC):
        sels[qd, qd * 128:(qd + 1) * 128] = 1.0
    sels = sels.astype(bf)

    qwv = np.asarray(q_norm_w, np.float32).reshape(128, 1)
    kwv = np.asarray(k_norm_w, np.float32).reshape(128, 1)

    woT = np.ascontiguousarray(np.asarray(wo, np.float32).T).astype(bf)

    xTb = []
    for b in range(2):
        xTb.append(np.ascontiguousarray(np.asarray(x[b], np.float32).T)
                   .astype(bf))

    wq = np.asarray(wqkv, np.float32)
    q_sz = NH * HD
    in_maps = []
    for c_id in range(N_CORES):
        b, rk = c_id // 4, c_id % 4
        rows = np.concatenate([
            wq[rk * HL * HD:(rk + 1) * HL * HD],          # 4 q heads
            wq[q_sz + rk * HD: q_sz + (rk + 1) * HD],     # k head
            wq[q_sz + NKV * HD + rk * HD:
               q_sz + NKV * HD + (rk + 1) * HD],          # v head
        ], axis=0)                                        # [768, 2048]
        wslice = np.ascontiguousarray(rows.T).astype(bf)  # [2048, 768]
        in_maps.append({
            "xT": xTb[b], "wslice": wslice, "woT": woT,
            "cosF": cosF, "sinF": sinF, "swapP": swapP,
            "ident": ident, "masks": masks, "onehots": oneh, "sels": sels,
            "qw": qwv, "kw": kwv,
        })
    return in_maps


_NC_CACHE = {}


def kernel(x, freqs_cis, mask, wqkv, wo, q_norm_w, k_norm_w):
    x = np.asarray(x)
    S = x.shape[1]
    if S not in _NC_CACHE:
        _NC_CACHE[S] = build_graph(S)
    nc = _NC_CACHE[S]
    in_maps = make_in_maps(x, freqs_cis, wqkv, wo, q_norm_w, k_norm_w, S)
    res = run_bass_kernel_spmd(nc, in_maps, core_ids=list(range(N_CORES)))
    TPT = S // 4
    out = np.empty((2, S, DIM), np.float32)
    for c_id in range(N_CORES):
        b, rk = c_id // 4, c_id % 4
        out[b, rk * TPT:(rk + 1) * TPT, :] = res.results[c_id]["out"].T.astype(np.float32)
    return out



# revision 1
# speedup vs baseline: 1.4526x; 1.4526x over previous
"""Distributed Trainium2 kernel for nn_Attention (dense transformer block:
fused QKV projection + per-head RMSNorm + rotary + causal GQA attention + output
projection), running SPMD on 8 NeuronCores.

Sharding (rank-uniform, no divergent control flow):
  - 8 cores = 2 batch groups x 4 tensor-parallel ranks.
  - Core c: batch b = c // 4, rank r = c % 4.
  - QKV projection + attention are head-sharded: core r computes q heads
    4r..4r+3 and kv head r for ALL tokens of its batch (wqkv column slice is
    per-core input data, so the compiled graph is identical on every core).
  - Every core runs the same causal tile sweep -> perfect load balance.
  - One AllToAll (2MB bf16) re-shards y from head-split to token-split, then
    the output projection runs locally with the full contraction dim
    (no all-reduce).

Layout tricks:
  - Host pre-transposes x, wqkv, wo so the kernel's matmuls need no on-device
    transposes (except tiny 128x128 PE transposes for V).
  - Scores are computed transposed [kv, q] so the softmax denominator comes
    from a ones-vector matmul on the TensorEngine and exp is fused into the
    PSUM->SBUF eviction on the ScalarEngine.
  - RMSNorm reduces to a per-token scalar (applied to q pre-matmul, and to k
    pre-matmul), computed with a ones-matmul over the squared tile; the
    1/sqrt(head_dim) score scale folds into the q-side scalar.
  - Rope's even/odd pair swap is a 128x128 permutation matmul in f32r.
  - All big matmuls run in bf16 with f32 PSUM accumulation.
"""

import numpy as np
import ml_dtypes

import concourse.bass as bass
import concourse.bass_isa as bass_isa
import concourse.mybir as mybir
import concourse.tile as tile
from concourse import bacc
from concourse.bass_utils import run_bass_kernel_spmd

BF16 = mybir.dt.bfloat16
F32 = mybir.dt.float32
F32R = mybir.dt.float32r

DIM = 2048
NH = 16
NKV = 4
HD = 128
EPS = 1e-5
N_CORES = 8
RG = [[0, 1, 2, 3], [4, 5, 6, 7]]  # per-batch tensor-parallel groups

HL = NH // NKV  # q heads per core (= GQA group size) = 4
EW = HL * HD + 2 * HD  # wqkv column-slice width per core = 768
NDT = DIM // 128  # contraction tiles = 16


def build_graph(S):
    """Build + compile the SPMD graph for sequence length S. Returns nc."""
    TPT = S // 4       # tokens per core after the A2A (output rows per core)
    TCW = S // 4       # token chunk width for phase 1 (moving dim <= 512)
    NTT = S // TCW     # number of token chunks = 4
    QC = 512           # attention q-chunk width
    KB = 128           # kv block size
    NQC = S // QC      # q chunks per head
    NB = S // 128      # 128-token blocks (for V layout)

    nc = bacc.Bacc("TRN2", target_bir_lowering=False, debug=False,
                   num_devices=N_CORES)

    # ---- DRAM I/O ----
    xT_d = nc.dram_tensor("xT", [DIM, S], BF16, kind="ExternalInput")
    w_d = nc.dram_tensor("wslice", [DIM, EW], BF16, kind="ExternalInput")
    wo_d = nc.dram_tensor("woT", [DIM, DIM], BF16, kind="ExternalInput")
    cos_d = nc.dram_tensor("cosF", [128, S], F32, kind="ExternalInput")
    sin_d = nc.dram_tensor("sinF", [128, S], F32, kind="ExternalInput")
    swp_d = nc.dram_tensor("swapP", [128, 128], F32, kind="ExternalInput")
    idn_d = nc.dram_tensor("ident", [128, 128], BF16, kind="ExternalInput")
    msk_d = nc.dram_tensor("masks", [KB, (QC // KB) * QC], BF16, kind="ExternalInput")
    oh_d = nc.dram_tensor("onehots", [128, (S // 512) * (S // 512)], BF16,
                          kind="ExternalInput")
    sel_d = nc.dram_tensor("sels", [S // 512, (S // 512) * 128], BF16,
                           kind="ExternalInput")
    qw_d = nc.dram_tensor("qw", [128, 1], F32, kind="ExternalInput")
    kw_d = nc.dram_tensor("kw", [128, 1], F32, kind="ExternalInput")
    out_d = nc.dram_tensor("out", [DIM, TPT], BF16, kind="ExternalOutput")

    with tile.TileContext(nc) as tc:
        with tc.tile_pool(name="const", bufs=1) as cpool, \
             tc.tile_pool(name="wq", bufs=1) as wpool, \
             tc.tile_pool(name="big", bufs=1) as bigpool, \
             tc.tile_pool(name="dram", bufs=1, space="DRAM") as dpool:

            # constants
            swp = cpool.tile([128, 128], F32R, tag="swp")
            nc.sync.dma_start(swp[:], swp_d[:].bitcast(F32R))
            idn = cpool.tile([128, 128], BF16, tag="idn")
            nc.sync.dma_start(idn[:], idn_d[:])
            msk = cpool.tile([KB, (QC // KB) * QC], BF16, tag="msk")
            nc.sync.dma_start(msk[:], msk_d[:])
            qw = cpool.tile([128, 1], F32, tag="qw")
            nc.sync.dma_start(qw[:], qw_d[:])
            kw = cpool.tile([128, 1], F32, tag="kw")
            nc.sync.dma_start(kw[:], kw_d[:])
            ones = cpool.tile([128, 1], BF16, tag="ones")
            nc.vector.memset(ones[:], 1.0)
            oneh = cpool.tile([128, NQC * NQC], BF16, tag="oneh")
            nc.sync.dma_start(oneh[:], oh_d[:])
            sel = cpool.tile([NQC, NQC * 128], BF16, tag="sel")
            nc.sync.dma_start(sel[:], sel_d[:])
            onec = cpool.tile([1, 128], BF16, tag="onec")
            nc.vector.memset(onec[:], 1.0)
            b0 = cpool.tile([128, 1], F32, tag="b0")
            nc.vector.memset(b0[:], 0.0)
            bq = cpool.tile([1, 1], F32, tag="bq")
            nc.vector.memset(bq[:], float(HD * EPS))
            bk = cpool.tile([1, 1], F32, tag="bk")
            nc.vector.memset(bk[:], float(EPS))

            # full wqkv slice, staged once: [128, dt*EW + e]
            # (one DMA per contraction tile so the first matmuls start early)
            w_sb = wpool.tile([128, NDT * EW], BF16, tag="w")

            # long-lived activations
            qT = bigpool.tile([128, HL * S], BF16, tag="qT")
            kT = bigpool.tile([128, S], BF16, tag="kT")
            V = bigpool.tile([128, S], BF16, tag="V")   # [tok%128, blk*128+d]
            yT = bigpool.tile([128, HL * S], BF16, tag="yT")

            # ---------------- Phase 1: QKV + norm + rope ----------------
            with tc.tile_pool(name="x", bufs=2) as xpool, \
                 tc.tile_pool(name="cs", bufs=2) as cspool, \
                 tc.tile_pool(name="scr", bufs=2) as scr, \
                 tc.tile_pool(name="smol", bufs=2) as smol, \
                 tc.tile_pool(name="p1", bufs=2, space="PSUM") as p1, \
                 tc.tile_pool(name="psw", bufs=2, space="PSUM") as psw, \
                 tc.tile_pool(name="pss", bufs=2, space="PSUM") as pss, \
                 tc.tile_pool(name="pvt", bufs=2, space="PSUM") as pvt:

                def process_qk(ps, et, tt, cos_t, sin_t):
                    is_q = et < HL
                    # squared tile (raw, pre-normweight) -> bf16
                    sqv = smol.tile([128, TCW], BF16, tag="sq2", name="sqv")
                    nc.scalar.activation(
                        sqv[:], ps[:],
                        mybir.ActivationFunctionType.Square, bias=b0[:])
                    ss = pss.tile([1, TCW], F32, tag="ss", name="ss")
                    nc.tensor.matmul(ss[:], ones[:], sqv[:],
                                     start=True, stop=True)
                    qf = scr.tile([128, TCW], F32R, tag="qf", name="qf")
                    nc.scalar.mul(qf[:], ps[:], (qw if is_q else kw)[:])
                    sq = smol.tile([1, TCW], F32, tag="sqs", name="sq")
                    if is_q:
                        nc.scalar.activation(
                            sq[:], ss[:],
                            mybir.ActivationFunctionType.Sqrt,
                            bias=bq[:], scale=1.0)
                    else:
                        nc.scalar.activation(
                            sq[:], ss[:],
                            mybir.ActivationFunctionType.Sqrt,
                            bias=bk[:], scale=1.0 / HD)
                    inv = smol.tile([1, TCW], F32, tag="inv", name="inv")
                    nc.vector.reciprocal_approx_fast(inv[:], sq[:])
                    invb = scr.tile([128, TCW], F32, tag="invb", name="invb")
                    nc.gpsimd.partition_broadcast(invb[:], inv[:])
                    sw = psw.tile([128, TCW], F32, tag="sw", name="sw")
                    nc.tensor.matmul(sw[:], swp[:], qf[:],
                                     start=True, stop=True)
                    t1 = scr.tile([128, TCW], F32, tag="t1", name="t1")
                    nc.vector.tensor_mul(t1[:], qf[:], cos_t[:])
                    t2 = scr.tile([128, TCW], F32, tag="t2", name="t2")
                    nc.vector.tensor_mul(t2[:], sw[:], sin_t[:])
                    nc.vector.tensor_add(t1[:], t1[:], t2[:])
                    dst = (qT[:, et * S + tt * TCW: et * S + tt * TCW + TCW]
                           if is_q else
                           kT[:, tt * TCW: tt * TCW + TCW])
                    nc.vector.tensor_mul(dst, t1[:], invb[:])

                def process_v(ps, tt):
                    vb = smol.tile([128, TCW], BF16, tag="vb", name="vb")
                    nc.scalar.copy(vb[:], ps[:])
                    for bb in range(TCW // 128):
                        tp = pvt.tile([128, 128], BF16, tag="tp", name="tp")
                        nc.tensor.transpose(
                            tp[:], vb[:, bb * 128:(bb + 1) * 128], idn[:])
                        blk = tt * (TCW // 128) + bb
                        nc.scalar.copy(V[:, blk * 128:(blk + 1) * 128], tp[:])

                pending = None  # (psum, et, tt, cos_t, sin_t)
                for tt in range(NTT):
                    xt = xpool.tile([128, NDT * TCW], BF16, tag="x")
                    for dt in range(NDT):
                        if tt == 0:  # interleave weight panels in need-order
                            nc.sync.dma_start(
                                w_sb[:, dt * EW:(dt + 1) * EW],
                                w_d[dt * 128:(dt + 1) * 128, :])
                        nc.scalar.dma_start(
                            xt[:, dt * TCW:(dt + 1) * TCW],
                            xT_d[dt * 128:(dt + 1) * 128,
                                 tt * TCW:(tt + 1) * TCW])
                    cos_t = cspool.tile([128, TCW], F32, tag="cos")
                    nc.sync.dma_start(cos_t[:], cos_d[:, tt * TCW:(tt + 1) * TCW])
                    sin_t = cspool.tile([128, TCW], F32, tag="sin")
                    nc.sync.dma_start(sin_t[:], sin_d[:, tt * TCW:(tt + 1) * TCW])

                    for et in range(HL + 2):
                        ps = p1.tile([128, TCW], F32, tag="ps")
                        for dt in range(NDT):
                            nc.tensor.matmul(
                                ps[:],
                                w_sb[:, dt * EW + et * 128:dt * EW + (et + 1) * 128],
                                xt[:, dt * TCW:(dt + 1) * TCW],
                                start=(dt == 0), stop=(dt == NDT - 1),
                            )
                        # process the PREVIOUS tile now: its cross-engine
                        # waits overlap this tile's matmul group
                        if pending is not None:
                            pps, pet, ptt, pc, psn_ = pending
                            if pet < HL + 1:
                                process_qk(pps, pet, ptt, pc, psn_)
                            else:
                                process_v(pps, ptt)
                        pending = (ps, et, tt, cos_t, sin_t)
                pps, pet, ptt, pc, psn_ = pending
                if pet < HL + 1:
                    process_qk(pps, pet, ptt, pc, psn_)
                else:
                    process_v(pps, ptt)

            # ---------------- Phase 2: causal attention ----------------
            with tc.tile_pool(name="exp", bufs=6) as epool, \
                 tc.tile_pool(name="rs", bufs=2) as rspool, \
                 tc.tile_pool(name="pa", bufs=2, space="PSUM") as pa, \
                 tc.tile_pool(name="py", bufs=1, space="PSUM") as py, \
                 tc.tile_pool(name="pn", bufs=1, space="PSUM") as pn, \
                 tc.tile_pool(name="pb", bufs=1, space="PSUM") as pb:

                # yf gathers the A2A'd y slices as they arrive, per head
                yf = bigpool.tile([128, NDT * TPT], BF16, tag="yf")
                pid = nc.gpsimd.partition_id()
                roff = (pid % 4) * TPT

                for h in range(HL):
                    # g-outer sweep: each kv block's kT/V stationary serves all
                    # active q-chunks before the PE loads the next weights.
                    ps_ys = [py.tile([128, QC], F32, tag=f"y{qc}",
                                     name=f"psy{qc}")
                             for qc in range(NQC)]
                    psn = pn.tile([NQC, QC], F32, tag="n")
                    for g in range(NB):
                        qcs = [qc for qc in range(NQC) if g < 4 * (qc + 1)]
                        exs = {}
                        for qc in qcs:
                            ps_s = pa.tile([KB, QC], F32, tag="s")
                            nc.tensor.matmul(
                                ps_s[:],
                                kT[:, g * KB:(g + 1) * KB],
                                qT[:, h * S + qc * QC: h * S + (qc + 1) * QC],
                                start=True, stop=True)
                            ex = epool.tile([KB, QC], BF16, tag="e")
                            nc.scalar.activation(
                                ex[:], ps_s[:],
                                mybir.ActivationFunctionType.Exp,
                                bias=b0[0:KB, :])
                            if qc == g // 4:  # diagonal region: causal mask
                                t = g % 4
                                nc.vector.tensor_mul(
                                    ex[:], ex[:], msk[:, t * QC:(t + 1) * QC])
                            exs[qc] = ex
                        for qc in qcs:
                            nc.tensor.matmul(
                                ps_ys[qc][:],
                                V[:, g * 128:(g + 1) * 128],
                                exs[qc][:],
                                start=(g == 0), stop=(g == 4 * qc + 3))
                        for qc in qcs:
                            nc.tensor.matmul(
                                psn[:], oneh[:, qc * NQC:(qc + 1) * NQC],
                                exs[qc][:],
                                start=(g == 0 and qc == 0),
                                stop=(g == NB - 1 and qc == NQC - 1))

                    rs4 = rspool.tile([NQC, QC], F32, tag="r")
                    nc.vector.reciprocal_approx_fast(rs4[:], psn[:])
                    rsb16 = rspool.tile([NQC, QC], BF16, tag="r16")
                    nc.vector.tensor_copy(rsb16[:], rs4[:])
                    for qd in range(NQC):
                        ps_b = pb.tile([128, QC], F32, tag="b")
                        nc.tensor.matmul(
                            ps_b[:], sel[:, qd * 128:(qd + 1) * 128],
                            rsb16[:], start=True, stop=True)
                        rsb = rspool.tile([128, QC], F32, tag="rb")
                        nc.scalar.copy(rsb[:], ps_b[:])
                        nc.vector.tensor_mul(
                            yT[:, h * S + qd * QC: h * S + (qd + 1) * QC],
                            ps_ys[qd][:], rsb[:])

                    # per-head AllGather of y, overlapped with the next head's
                    # attention; each core reads back its token quarter via a
                    # partition_id-derived dynamic column offset.
                    in_b = dpool.tile([128, S], BF16, tag=f"agin{h}")
                    out_b = dpool.tile([4 * 128, S], BF16, tag=f"agout{h}")
                    nc.scalar.dma_start(in_b[:], yT[:, h * S:(h + 1) * S])
                    nc.gpsimd.collective_compute(
                        "AllGather", mybir.AluOpType.bypass,
                        replica_groups=RG,
                        ins=[in_b.opt()], outs=[out_b.opt()])
                    for r in range(4):
                        et = 4 * r + h
                        nc.gpsimd.dma_start(
                            yf[:, et * TPT:(et + 1) * TPT],
                            out_b[r * 128:(r + 1) * 128, bass.ds(roff, TPT)])

            # ---------------- output projection (two passes) ----------------
            # Pass A accumulates heads 0..2 (available before the last
            # AllGather) into bf16 partials; pass B adds head 3's contribution
            # as soon as its gather lands. woT streamed per output tile.
            with tc.tile_pool(name="wos", bufs=4) as wopool, \
                 tc.tile_pool(name="part", bufs=1) as partpool, \
                 tc.tile_pool(name="ot", bufs=2) as otpool, \
                 tc.tile_pool(name="po", bufs=2, space="PSUM") as po:
                part = partpool.tile([128, NDT * TPT], BF16, tag="part")
                etsA = [4 * r + hl for hl in range(HL - 1) for r in range(4)]
                etsB = [4 * r + (HL - 1) for r in range(4)]
                for ot in range(NDT):
                    wos = wopool.tile([128, NDT * 128], BF16, tag="wos")
                    nc.sync.dma_start(
                        wos[:].rearrange("p (a o) -> p a o", a=NDT),
                        wo_d[:, ot * 128:(ot + 1) * 128]
                            .rearrange("(a p) o -> p a o", p=128))
                    ps_o = po.tile([128, TPT], F32, tag="o")
                    for i, et in enumerate(etsA):
                        nc.tensor.matmul(
                            ps_o[:],
                            wos[:, et * 128:(et + 1) * 128],
                            yf[:, et * TPT:(et + 1) * TPT],
                            start=(i == 0), stop=(i == len(etsA) - 1))
                    nc.scalar.copy(part[:, ot * TPT:(ot + 1) * TPT], ps_o[:])
                for ot in range(NDT):
                    wosb = wopool.tile([128, 4 * 128], BF16, tag="wosb")
                    for i, et in enumerate(etsB):
                        nc.sync.dma_start(
                            wosb[:, i * 128:(i + 1) * 128],
                            wo_d[et * 128:(et + 1) * 128,
                                 ot * 128:(ot + 1) * 128])
                    ps_o = po.tile([128, TPT], F32, tag="o")
                    for i, et in enumerate(etsB):
                        nc.tensor.matmul(
                            ps_o[:],
                            wosb[:, i * 128:(i + 1) * 128],
                            yf[:, et * TPT:(et + 1) * TPT],
                            start=(i == 0), stop=(i == len(etsB) - 1))
                    ott = otpool.tile([128, TPT], BF16, tag="ot")
                    nc.vector.tensor_add(ott[:], ps_o[:],
                                         part[:, ot * TPT:(ot + 1) * TPT])
                    nc.sync.dma_start(out_d[ot * 128:(ot + 1) * 128, :], ott[:])

    nc.compile()
    return nc


def make_in_maps(x, freqs_cis, wqkv, wo, q_norm_w, k_norm_w, S):
    """Host-side sharding / layout prep. Returns list of 8 input dicts."""
    bf = ml_dtypes.bfloat16
    QC = 512
    KB = 128

    # rope tables: [128, S]; row 2i & 2i+1 carry cos[t, i]; sin signed
    cos = np.asarray(freqs_cis[:S, :, 0], np.float32)   # [S, 64]
    sin = np.asarray(freqs_cis[:S, :, 1], np.float32)
    cosF = np.repeat(cos.T, 2, axis=0).astype(np.float32)      # [128, S]
    sinF = np.repeat(sin.T, 2, axis=0).astype(np.float32)
    sinF[0::2] *= -1.0
    cosF = np.ascontiguousarray(cosF)
    sinF = np.ascontiguousarray(sinF)

    swapP = np.zeros((128, 128), np.float32)
    for i in range(64):
        swapP[2 * i, 2 * i + 1] = 1.0
        swapP[2 * i + 1, 2 * i] = 1.0
    ident = np.eye(128, dtype=bf)

    # masks [KB, (QC//KB)*QC]: pattern t for the t-th kv block inside the
    # diagonal QC-region: allowed iff (t*KB + r) <= c
    r = np.arange(KB)[:, None]
    c = np.arange(QC)[None, :]
    pats = [((t * KB + r) <= c).astype(np.float32) for t in range(QC // KB)]
    masks = np.concatenate(pats, axis=1).astype(bf)

    NQC = S // 512
    oneh = np.zeros((128, NQC * NQC), np.float32)
    for qc in range(NQC):
        oneh[:, qc * NQC + qc] = 1.0
    oneh = oneh.astype(bf)
    sels = np.zeros((NQC, NQC * 128), np.float32)
    for qd in range(NQC):
        sels[qd, qd * 128:(qd + 1) * 128] = 1.0
    sels = sels.astype(bf)

    qwv = np.asarray(q_norm_w, np.float32).reshape(128, 1)
    kwv = np.asarray(k_norm_w, np.float32).reshape(128, 1)

    woT = np.ascontiguousarray(np.asarray(wo, np.float32).T).astype(bf)

    xTb = []
    for b in range(2):
        xTb.append(np.ascontiguousarray(np.asarray(x[b], np.float32).T)
                   .astype(bf))

    wq = np.asarray(wqkv, np.float32)
    q_sz = NH * HD
    in_maps = []
    for c_id in range(N_CORES):
        b, rk = c_id // 4, c_id % 4
        rows = np.concatenate([
            wq[rk * HL * HD:(rk + 1) * HL * HD],          # 4 q heads
            wq[q_sz + rk * HD: q_sz + (rk + 1) * HD],     # k head
            wq[q_sz + NKV * HD + rk * HD:
               q_sz + NKV * HD + (rk + 1) * HD],          # v head
        ], axis=0)                                        # [768, 2048]
        wslice = np.ascontiguousarray(rows.T).astype(bf)  # [2048, 768]
        in_maps.append({
            "xT": xTb[b], "wslice": wslice, "woT": woT,
            "cosF": cosF, "sinF": sinF, "swapP": swapP,
            "ident": ident, "masks": masks, "onehots": oneh, "sels": sels,
            "qw": qwv, "kw": kwv,
        })
    return in_maps


_NC_CACHE = {}


def kernel(x, freqs_cis, mask, wqkv, wo, q_norm_w, k_norm_w):
    x = np.asarray(x)
    S = x.shape[1]
    if S not in _NC_CACHE:
        _NC_CACHE[S] = build_graph(S)
    nc = _NC_CACHE[S]
    in_maps = make_in_maps(x, freqs_cis, wqkv, wo, q_norm_w, k_norm_w, S)
    res = run_bass_kernel_spmd(nc, in_maps, core_ids=list(range(N_CORES)))
    TPT = S // 4
    out = np.empty((2, S, DIM), np.float32)
    for c_id in range(N_CORES):
        b, rk = c_id // 4, c_id % 4
        out[b, rk * TPT:(rk + 1) * TPT, :] = res.results[c_id]["out"].T.astype(np.float32)
    return out

